# revision 22
# baseline (speedup 1.0000x reference)
"""
Trainium2 Bass kernel for DirectRankingModel:
    h = tanh(x @ W1.T + b1); s = (h @ W2.T + b2); e = exp(s)
    out = e / segment_sum(e, T)[T]    with 2 segments, N = 2,000,000 rows.

Strategy (8 NeuronCores, data-parallel over rows; v2):
  - Host: cast x to f16 and block-transpose into chunks of 8192 rows:
    xd[ch] = [128, 4096] where partitions 0-63 hold features of rows
    [0,4096) and 64-127 hold rows [4096,8192) -> HWDGE DMA (1 MiB each),
    row-split PE runs both halves concurrently.
  - 62 "megas" (253952 rows) per core: only 1.6% padding (vs 4.9%).
  - mm1: K=64 f16 matmuls, N=512, pairs on partition halves -> PSUM
    ph [128, 1536] (3 banks, 2 bufs).
  - tanh on ACT in 1536-wide instructions (PSUM->SBUF f16) -- ACT is the
    kernel's roofline (~0.95 ns/col/core).
  - mm2: 32-strip trick; block b -> psum partition b%128, strips cycle
    c=b%32 with col-group tile_position; exp per 128-block super-tile.
  - Segment sums: single sel mask (1.0 where T==1) + per-core pad
    correction scalars; per-ST incremental DVE sums; ones-matmul
    partition reduce; 2-float HBM AllReduce across cores.
  - normalize: out = E * (inv0 + sel*(inv1-inv0)) and one 1 MiB store.
"""

import os
import sys

import numpy as np

for _p in ("/opt/trn_rl_repo", "/root/.axon_site/_ro/trn_rl_repo"):
    if os.path.isdir(_p) and _p not in sys.path:
        sys.path.insert(0, _p)

import concourse.bacc as bacc
import concourse.bass as bass
import concourse.tile as tile
from concourse import mybir
from concourse.bass_utils import run_bass_kernel_spmd

F16 = mybir.dt.float16
F32 = mybir.dt.float32
ALU = mybir.AluOpType
ACTF = mybir.ActivationFunctionType

N_CORES = 8
N_ROWS = 2_000_000
IN_DIM = 64
HID = 128

# Device-side geometry (per core).
Q = 256                     # rows per score-block
CH_ROWS = 8192              # rows per DMA chunk ([128, 4096] f16 = 1 MiB)
N_CH = 31                   # chunks per core
R_CORE = N_CH * CH_ROWS     # 253952 rows per core
N_PAD = N_CORES * R_CORE    # 2031616 rows total (padded)
MM_PER_CH = 16              # mm1 matmuls (512 rows each) per chunk
N_MM = N_CH * MM_PER_CH     # 496
PH_MMS = 3                  # mm1 outputs per PSUM tile -> ACT N=1536
PH_COLS = 512 * PH_MMS
BLK_PER_HT = PH_COLS // Q   # 6 score blocks per ht tile
N_BLK = R_CORE // Q         # 992
N_ST = (N_BLK + 127) // 128  # 8 super-tiles (last partial: 96 blocks)
COLS = N_ST * Q             # 2048 e/sel/out columns per partition

# DVE-assisted tanh: every ASSIST_EVERY-th ht tile is computed on the
# vector engine with a minimax R(5,4) rational (offloads ~13% of the
# ACT roofline).  tanh(z) ~ z*(t^2 + A2 t + A0)/(t^2 + B2 t + B0), t=z^2,
# fit on |z| <= 4.8 (f16 end-to-end max err ~3e-3).
ASSIST_EVERY = 10**9        # disabled: DVE tanh chains stall the PE stream
ASSIST_PHASE = -1
MM2_DEFER = 2               # ht tiles between tanh and its mm2 emission
A2, A0 = -97.93266143005741, -1801.597436686564
B2, B0 = -680.3934051370769, -1811.5100012510918


def _ap(handle_ap, offset, dims):
    """Custom access pattern on a DRAM tensor: dims = [[step, count], ...]."""
    return bass.AP(tensor=handle_ap.tensor, offset=offset, ap=list(dims))


def build_nc(n_cores=N_CORES, use_coll=True, stage=9):
    """Build the per-core Bass program (SPMD: same program, sliced inputs)."""
    from contextlib import ExitStack

    nc = bacc.Bacc(num_devices=n_cores)

    x_in = nc.declare_dram_parameter("x", [N_CH, 128, 4096], F16, isOutput=False)
    sel_in = nc.declare_dram_parameter("sel", [128 * COLS], F32, isOutput=False)
    w1t_in = nc.declare_dram_parameter("w1t", [IN_DIM, HID], F16, isOutput=False)
    w2s_in = nc.declare_dram_parameter("w2s", [HID, 32 * 32], F16, isOutput=False)
    b1_in = nc.declare_dram_parameter("b1", [HID], F32, isOutput=False)
    b2_in = nc.declare_dram_parameter("b2", [1], F32, isOutput=False)
    padc_in = nc.declare_dram_parameter("padc", [2], F32, isOutput=False)
    out_t = nc.declare_dram_parameter("out", [128 * COLS], F32, isOutput=True)
    gs_t = nc.declare_dram_parameter("gsums", [2], F32, isOutput=True)

    cc_in = nc.dram_tensor("cc_in", [2], F32)
    cc_out = nc.dram_tensor("cc_out", [2], F32, addr_space="Shared")

    with ExitStack() as ctx:
        tc = ctx.enter_context(tile.TileContext(nc))
        singles = ctx.enter_context(tc.tile_pool(name="singles", bufs=1))
        xx_pool = ctx.enter_context(tc.tile_pool(name="xx", bufs=3))
        ht_pool = ctx.enter_context(tc.tile_pool(name="ht", bufs=MM2_DEFER + 4))
        dv_pool = ctx.enter_context(tc.tile_pool(name="dv", bufs=1))
        ph_pool = ctx.enter_context(tc.tile_pool(name="ph", bufs=2, space="PSUM"))
        ps_pool = ctx.enter_context(tc.tile_pool(name="ps", bufs=1, space="PSUM"))

        # ---- static setup ----------------------------------------------
        # First x chunk before anything else on the HWDGE queue; a small
        # leading slice so the first matmul can start ~5us earlier.
        xx_tiles = {}
        xx_tiles[0] = xx_pool.tile([128, 4096], F16, tag="xx", name="xx")
        nc.sync.dma_start(
            out=xx_tiles[0][:, 0:512],
            in_=_ap(x_in[:], 0, [[4096, 128], [1, 512]]),
        )
        w1t_sb = singles.tile([128, HID], F16)     # both halves hold W1T
        nc.sync.dma_start(
            out=w1t_sb[:],
            in_=_ap(w1t_in[:], 0, [[0, 2], [HID, IN_DIM], [1, HID]]),
        )
        b1_sb = singles.tile([128, 1], F32)
        nc.sync.dma_start(out=b1_sb[:], in_=_ap(b1_in[:], 0, [[1, HID], [1, 1]]))
        b2_sb = singles.tile([128, 1], F32)
        nc.sync.dma_start(out=b2_sb[:], in_=_ap(b2_in[:], 0, [[0, 128], [1, 1]]))
        nc.sync.dma_start(
            out=xx_tiles[0][:, 512:4096],
            in_=_ap(x_in[:], 512, [[4096, 128], [1, 3584]]),
        )
        xx_tiles[1] = xx_pool.tile([128, 4096], F16, tag="xx", name="xx")
        nc.sync.dma_start(
            out=xx_tiles[1][:],
            in_=_ap(x_in[:], 128 * 4096, [[4096, 128], [1, 4096]]),
        )

        # 32 strip matrices [128, 32] fp16, strip c has W2 in column c.
        strips = singles.tile([128, 32, 32], F16)
        nc.sync.dma_start(
            out=strips[:], in_=_ap(w2s_in[:], 0, [[32 * 32, HID], [1, 32 * 32]])
        )
        pc_sb = singles.tile([1, 2], F32)
        nc.sync.dma_start(out=pc_sb[:], in_=_ap(padc_in[:], 0, [[2, 1], [1, 2]]))

        sel_sb = singles.tile([128, COLS], F32)
        nc.sync.dma_start(
            out=sel_sb[:], in_=_ap(sel_in[:], 0, [[COLS, 128], [1, COLS]])
        )

        e_sb = singles.tile([128, COLS], F32)
        out_sb = singles.tile([128, COLS], F32)
        scr = singles.tile([128, Q], F32)
        rr_sel = singles.tile([128, N_ST], F32)
        rr_tot = singles.tile([128, N_ST], F32)
        rr2 = singles.tile([128, 2], F32)
        rr_red = singles.tile([128, 2], F32)
        ones_sb = singles.tile([128, 1], F32)
        tiny = singles.tile([128, 1], F32)
        g2 = singles.tile([1, 2], F32)
        g_sb = singles.tile([128, 2], F32)
        inv = singles.tile([128, 2], F32)
        dinv = singles.tile([128, 1], F32)

        nc.vector.memset(ones_sb[:], 1.0)
        # Zero the unused corner of E (last super-tile has 96 blocks).
        nc.vector.memset(e_sb[96:128, (N_ST - 1) * Q : N_ST * Q], 0.0)
        # Dummy activation: pulls ACT_TABLE_LOAD off the critical path.
        nc.scalar.activation(
            out=tiny[:], in_=ones_sb[:], func=ACTF.Tanh, bias=0.0, scale=1.0
        )
        if use_coll:
            # Warmup AllReduce: pulls the CC-path cold start off the tail
            # and aligns the cores early.
            warm = singles.tile([1, 2], F32)
            cc_win = nc.dram_tensor("cc_win", [2], F32)
            cc_wout = nc.dram_tensor("cc_wout", [2], F32, addr_space="Shared")
            nc.vector.memset(warm[:], 0.0)
            nc.gpsimd.dma_start(out=cc_win[:], in_=warm[0:1, :])
            nc.gpsimd.collective_compute(
                "AllReduce",
                ALU.add,
                replica_groups=[list(range(n_cores))],
                ins=[cc_win[:]],
                outs=[cc_wout[:]],
            )

        # ---- phase 1: mm1 / tanh / mm2 / exp ---------------------------
        # Global mm1 index m: chunk ch=m//16, j=m%16 -> half=j%2, t=j//2.
        # R-order rows [512m, 512(m+1)) = chunk rows half*4096 + 512t ...
        ph = None
        ht = None
        s_ps = None
        next_blk = 0
        cur_tile = 0
        pending_sums = []  # (st, tile_at_exp_emission)

        def emit_st_sums(st):
            # incremental segment sums for super-tile st (DVE)
            nc.vector.tensor_mul(
                scr[:], e_sb[:, st * Q : (st + 1) * Q],
                sel_sb[:, st * Q : (st + 1) * Q],
            )
            nc.vector.reduce_sum(
                rr_sel[:, st : st + 1], scr[:], axis=mybir.AxisListType.X
            )
            nc.vector.reduce_sum(
                rr_tot[:, st : st + 1],
                e_sb[:, st * Q : (st + 1) * Q],
                axis=mybir.AxisListType.X,
            )

        def flush_sums(margin_tile):
            # Only emit DVE sum ops whose exp is comfortably past: keeps
            # the DVE FIFO from blocking tanh chains behind a sum that
            # still waits on ACT/PE progress.
            while pending_sums and pending_sums[0][1] <= margin_tile:
                st, _ = pending_sums.pop(0)
                emit_st_sums(st)

        def emit_mm2_upto(bmax):
            nonlocal s_ps, next_blk
            while next_blk < bmax:
                b = next_blk
                c = b % 32
                g = (b % 128) // 32
                if b % 128 == 0:
                    s_ps = ps_pool.tile([128, Q], F32, tag="score")
                k = b // BLK_PER_HT
                hoff = Q * (b % BLK_PER_HT)
                nc.tensor.matmul(
                    s_ps[32 * g : 32 * g + 32, :],
                    strips[:, c, :],
                    ht_tiles[k][:, hoff : hoff + Q],
                    start=(c == 0),
                    stop=(c == 31),
                    skip_group_check=True,
                    tile_position=(0, 32 * g),
                )
                next_blk += 1
                if next_blk % 128 == 0 or next_blk == N_BLK:
                    st = (next_blk - 1) // 128
                    npart = 128 if next_blk % 128 == 0 else (next_blk - st * 128)
                    nc.scalar.activation(
                        out=e_sb[0:npart, st * Q : (st + 1) * Q],
                        in_=s_ps[0:npart, :],
                        func=ACTF.Exp,
                        bias=b2_sb[0:npart, :],
                        scale=1.0,
                    )
                    pending_sums.append((st, cur_tile))

        ht_tiles = {}
        for m in range(N_MM):
            ch, j = divmod(m, MM_PER_CH)
            if j == 4 and ch + 2 < N_CH and (ch + 2) not in xx_tiles:
                ch2 = ch + 2
                xx_tiles[ch2] = xx_pool.tile([128, 4096], F16, tag="xx", name="xx")
                nc.sync.dma_start(
                    out=xx_tiles[ch2][:],
                    in_=_ap(
                        x_in[:], ch2 * 128 * 4096, [[4096, 128], [1, 4096]]
                    ),
                )
            if j == 0 and ch not in xx_tiles:
                xx_tiles[ch] = xx_pool.tile([128, 4096], F16, tag="xx", name="xx")
                nc.sync.dma_start(
                    out=xx_tiles[ch][:],
                    in_=_ap(
                        x_in[:], ch * 128 * 4096, [[4096, 128], [1, 4096]]
                    ),
                )
            half, t = j % 2, j // 2
            slot = m % PH_MMS
            if slot == 0:
                ph = ph_pool.tile([128, PH_COLS], F32, tag="ph")
            nc.tensor.matmul(
                ph[:, 512 * slot : 512 * (slot + 1)],
                w1t_sb[64 * half : 64 * half + 64, :],
                xx_tiles[ch][64 * half : 64 * half + 64, 512 * t : 512 * (t + 1)],
                start=True,
                stop=True,
            )
            if slot == PH_MMS - 1 or m == N_MM - 1:
                k = m // PH_MMS
                cur_tile = k
                flush_sums(k - 3)
                ncols = 512 * (slot + 1)
                ht_tiles[k] = ht_pool.tile([128, PH_COLS], F16, tag="ht", name="ht")
                hk = ht_tiles[k]
                if k % ASSIST_EVERY == ASSIST_PHASE and ncols == PH_COLS:
                    # DVE rational tanh: z=ph+b1; t=z^2; t4=t^2;
                    # ht = z*(t4 + A2 t + A0) / (t4 + B2 t + B0)
                    zs = dv_pool.tile([128, PH_COLS], F16, tag="dv_zs", name="zs")
                    tq = dv_pool.tile([128, PH_COLS], F16, tag="dv_t", name="tq")
                    t4 = dv_pool.tile([128, PH_COLS], F16, tag="dv_t4", name="t4")
                    nm = dv_pool.tile([128, PH_COLS], F16, tag="dv_nm", name="nm")
                    d32 = dv_pool.tile([128, PH_COLS], F32, tag="dv_d", name="d32")
                    r32 = dv_pool.tile([128, PH_COLS], F32, tag="dv_r", name="r32")
                    with nc.allow_low_precision(reason="f16 rational tanh"):
                        nc.vector.tensor_scalar(
                            out=zs[:], in0=ph[:], scalar1=b1_sb[:],
                            scalar2=None, op0=ALU.add,
                        )
                        nc.vector.tensor_mul(tq[:], zs[:], zs[:])
                        nc.vector.tensor_mul(t4[:], tq[:], tq[:])
                        nc.vector.tensor_scalar(
                            out=nm[:], in0=tq[:], scalar1=float(A2),
                            scalar2=float(A0), op0=ALU.mult, op1=ALU.add,
                        )
                        nc.vector.tensor_add(nm[:], nm[:], t4[:])
                        nc.vector.tensor_mul(nm[:], nm[:], zs[:])
                        nc.vector.tensor_scalar(
                            out=tq[:], in0=tq[:], scalar1=float(B2),
                            scalar2=float(B0), op0=ALU.mult, op1=ALU.add,
                        )
                        nc.vector.tensor_add(d32[:], tq[:], t4[:])
                        nc.vector.reciprocal_approx_fast(out=r32[:], in_=d32[:])
                        nc.vector.tensor_copy(t4[:], r32[:])
                        nc.vector.tensor_mul(hk[:], nm[:], t4[:])
                else:
                    nc.scalar.activation(
                        out=hk[:, 0:ncols],
                        in_=ph[:, 0:ncols],
                        func=ACTF.Tanh,
                        bias=b1_sb[:],
                        scale=1.0,
                    )
                if k >= MM2_DEFER:
                    emit_mm2_upto(((k - MM2_DEFER + 1) * PH_COLS) // Q)
        emit_mm2_upto(N_BLK)
        flush_sums(10**9)

        # ---- segment sums + allreduce ----------------------------------
        if stage <= 1:
            nc.sync.dma_start(
                out=_ap(out_t[:], 0, [[COLS, 128], [1, COLS]]), in_=e_sb[:]
            )
            nc.sync.dma_start(out=gs_t[:], in_=e_sb[0:1, 0:2])
            nc.compile()
            return nc

        # rr_sel/rr_tot [128, 8] -> rr2 [128, 2] -> partition reduce -> [1,2]
        nc.vector.reduce_sum(rr2[:, 0:1], rr_tot[:], axis=mybir.AxisListType.X)
        nc.vector.reduce_sum(rr2[:, 1:2], rr_sel[:], axis=mybir.AxisListType.X)
        ps_rr = ps_pool.tile([128, 2], F32, tag="score")
        nc.tensor.matmul(ps_rr[0:1, :], ones_sb[:], rr2[:], start=True, stop=True)
        nc.scalar.activation(
            out=rr_red[0:1, :], in_=ps_rr[0:1, :], func=ACTF.Copy, bias=0.0,
            scale=1.0,
        )
        # g2 = [sum0, sum1]: sum1 = sel_sum - padc[1];
        #                    sum0 = (tot - sel_sum) - padc[0]
        nc.vector.tensor_sub(g2[0:1, 1:2], rr_red[0:1, 1:2], pc_sb[0:1, 1:2])
        nc.vector.tensor_sub(g2[0:1, 0:1], rr_red[0:1, 0:1], rr_red[0:1, 1:2])
        nc.vector.tensor_sub(g2[0:1, 0:1], g2[0:1, 0:1], pc_sb[0:1, 0:1])

        if use_coll:
            nc.gpsimd.dma_start(out=cc_in[:], in_=g2[0:1, :])
            nc.gpsimd.collective_compute(
                "AllReduce",
                ALU.add,
                replica_groups=[list(range(n_cores))],
                ins=[cc_in[:]],
                outs=[cc_out[:]],
            )
            nc.sync.dma_start(out=gs_t[:], in_=cc_out[:])
            nc.sync.dma_start(
                out=g_sb[:], in_=_ap(cc_out[:], 0, [[0, 128], [1, 2]])
            )
        else:
            nc.sync.dma_start(out=gs_t[:], in_=g2[0:1, :])
            nc.sync.dma_start(
                out=g_sb[:], in_=_ap(gs_t[:], 0, [[0, 128], [1, 2]])
            )

        # ---- normalize + store -----------------------------------------
        nc.vector.reciprocal(out=inv[:], in_=g_sb[:])
        nc.vector.tensor_sub(dinv[:], inv[:, 1:2], inv[:, 0:1])
        nc.vector.tensor_scalar(
            out=out_sb[:],
            in0=sel_sb[:],
            scalar1=dinv[:],
            scalar2=inv[:, 0:1],
            op0=ALU.mult,
            op1=ALU.add,
        )
        nc.vector.tensor_mul(out_sb[:], out_sb[:], e_sb[:])
        nc.sync.dma_start(
            out=_ap(out_t[:], 0, [[COLS, 128], [1, COLS]]), in_=out_sb[:]
        )

    nc.compile()
    return nc


_NC_CACHE = {}


def _get_nc(key=0):
    if key not in _NC_CACHE:
        _NC_CACHE[key] = build_nc()
    return _NC_CACHE[key]


# Host-side index map (identical for every core): for actual row a in
# [0, R_CORE): position in the device (p, col) layout.
_IDX_CACHE = {}


def _layout_index():
    """Return (p, col) arrays mapping core-row a -> device layout slot."""
    if "idx" not in _IDX_CACHE:
        a = np.arange(R_CORE, dtype=np.int64)
        ch, o = a // CH_ROWS, a % CH_ROWS
        half, w = o // 4096, o % 4096
        t, jr = w // 512, w % 512
        m = MM_PER_CH * ch + 2 * t + half
        q = 512 * m + jr
        b, r = q // Q, q % Q
        p = b % 128
        col = (b // 128) * Q + r
        _IDX_CACHE["idx"] = (p, col)
    return _IDX_CACHE["idx"]


def prep_inputs(x, T, W1, b1, W2, b2, n_cores=N_CORES):
    """Host-side shard/layout prep -> per-core input maps."""
    n_rows = x.shape[0]
    assert n_rows == N_ROWS

    x = np.asarray(x, dtype=np.float32)
    xh = np.zeros((N_PAD, IN_DIM), dtype=np.float16)
    xh[:n_rows] = x
    # [n_chunk_tot, 2, 4096, 64] -> [n_chunk_tot, 2, 64, 4096] -> [., 128, 4096]
    n_ch_tot = N_PAD // CH_ROWS
    xd = np.ascontiguousarray(
        xh.reshape(n_ch_tot, 2, 4096, IN_DIM)
        .transpose(0, 1, 3, 2)
        .reshape(n_ch_tot, 128, 4096)
    )

    T = np.asarray(T)
    p_idx, c_idx = _layout_index()

    w1t = np.ascontiguousarray(np.asarray(W1, np.float32).T).astype(np.float16)
    w2s = np.zeros((HID, 32, 32), dtype=np.float16)
    w2v = np.asarray(W2, np.float32).reshape(HID).astype(np.float16)
    for c in range(32):
        w2s[:, c, c] = w2v
    w2s = w2s.reshape(HID, 32 * 32)
    b1h = np.asarray(b1, np.float32).reshape(HID).copy()
    b2h = np.asarray(b2, np.float32).reshape(1).copy()

    # Host estimate of the device's E value on padded rows (x = 0).
    h_pad = np.tanh(b1h)
    e_pad = float(np.exp(b2h[0] + np.asarray(W2, np.float32).reshape(HID) @ h_pad))

    in_maps = []
    for cid in range(n_cores):
        base = cid * R_CORE
        n_real = min(max(n_rows - base, 0), R_CORE)
        selv = np.zeros(R_CORE, dtype=np.float32)
        selv[:n_real] = T[base : base + n_real] == 1
        sel_layout = np.zeros((128, COLS), dtype=np.float32)
        sel_layout[p_idx, c_idx] = selv
        n_pad_c = R_CORE - n_real
        padc = np.array([n_pad_c * e_pad, 0.0], dtype=np.float32)
        in_maps.append(
            {
                "x": xd[cid * N_CH : (cid + 1) * N_CH],
                "sel": sel_layout.reshape(-1),
                "w1t": w1t,
                "w2s": w2s,
                "b1": b1h,
                "b2": b2h,
                "padc": padc,
            }
        )
    return in_maps


def run(x, T, W1, b1, W2, b2, trace=False):
    in_maps = prep_inputs(x, T, W1, b1, W2, b2)
    nc = _get_nc()
    res = run_bass_kernel_spmd(nc, in_maps, list(range(N_CORES)), trace=trace)
    p_idx, c_idx = _layout_index()
    n_rows = x.shape[0]
    out = np.empty(n_rows, dtype=np.float32)
    for cid in range(N_CORES):
        base = cid * R_CORE
        n_real = min(max(n_rows - base, 0), R_CORE)
        if n_real <= 0:
            break
        lay = res.results[cid]["out"].reshape(128, COLS)
        out[base : base + n_real] = lay[p_idx[:n_real], c_idx[:n_real]]
    return out, res


def kernel(x, T, W1, b1, W2, b2):
    out, _ = run(x, T, W1, b1, W2, b2)
    return out


# revision 23
# speedup vs baseline: 1.3515x; 1.3515x over previous
"""
Trainium2 Bass kernel for DirectRankingModel:
    h = tanh(x @ W1.T + b1); s = (h @ W2.T + b2); e = exp(s)
    out = e / segment_sum(e, T)[T]    with 2 segments, N = 2,000,000 rows.

Strategy (8 NeuronCores, data-parallel over rows; v2):
  - Host: cast x to f16 and block-transpose into chunks of 8192 rows:
    xd[ch] = [128, 4096] where partitions 0-63 hold features of rows
    [0,4096) and 64-127 hold rows [4096,8192) -> HWDGE DMA (1 MiB each),
    row-split PE runs both halves concurrently.
  - 62 "megas" (253952 rows) per core: only 1.6% padding (vs 4.9%).
  - mm1: K=64 f16 matmuls, N=512, pairs on partition halves -> PSUM
    ph [128, 1536] (3 banks, 2 bufs).
  - tanh on ACT in 1536-wide instructions (PSUM->SBUF f16) -- ACT is the
    kernel's roofline (~0.95 ns/col/core).
  - mm2: 32-strip trick; block b -> psum partition b%128, strips cycle
    c=b%32 with col-group tile_position; exp per 128-block super-tile.
  - Segment sums: single sel mask (1.0 where T==1) + per-core pad
    correction scalars; per-ST incremental DVE sums; ones-matmul
    partition reduce; 2-float HBM AllReduce across cores.
  - normalize: out = E * (inv0 + sel*(inv1-inv0)) and one 1 MiB store.
"""

import os
import sys

import numpy as np

for _p in ("/opt/trn_rl_repo", "/root/.axon_site/_ro/trn_rl_repo"):
    if os.path.isdir(_p) and _p not in sys.path:
        sys.path.insert(0, _p)

import concourse.bacc as bacc
import concourse.bass as bass
import concourse.tile as tile
from concourse import mybir
from concourse.bass_utils import run_bass_kernel_spmd

F16 = mybir.dt.float16
F32 = mybir.dt.float32
ALU = mybir.AluOpType
ACTF = mybir.ActivationFunctionType

N_CORES = 8
N_ROWS = 2_000_000
IN_DIM = 64
HID = 128

# Device-side geometry (per core).
Q = 256                     # rows per score-block
CH_ROWS = 8192              # rows per DMA chunk ([128, 4096] f16 = 1 MiB)
N_CH = 31                   # chunks per core
R_CORE = N_CH * CH_ROWS     # 253952 rows per core
N_PAD = N_CORES * R_CORE    # 2031616 rows total (padded)
MM_PER_CH = 16              # mm1 matmuls (512 rows each) per chunk
N_MM = N_CH * MM_PER_CH     # 496
PH_MMS = 3                  # mm1 outputs per PSUM tile -> ACT N=1536
PH_COLS = 512 * PH_MMS
BLK_PER_HT = PH_COLS // Q   # 6 score blocks per ht tile
N_BLK = R_CORE // Q         # 992
N_ST = (N_BLK + 127) // 128  # 8 super-tiles (last partial: 96 blocks)
COLS = N_ST * Q             # 2048 e/sel/out columns per partition

# DVE-assisted tanh: every ASSIST_EVERY-th ht tile is computed on the
# vector engine with a minimax R(5,4) rational (offloads ~13% of the
# ACT roofline).  tanh(z) ~ z*(t^2 + A2 t + A0)/(t^2 + B2 t + B0), t=z^2,
# fit on |z| <= 4.8 (f16 end-to-end max err ~3e-3).
ASSIST_EVERY = 10**9        # disabled: DVE tanh chains stall the PE stream
ASSIST_PHASE = -1
MM2_DEFER = 2               # ht tiles between tanh and its mm2 emission
A2, A0 = -97.93266143005741, -1801.597436686564
B2, B0 = -680.3934051370769, -1811.5100012510918


def _ap(handle_ap, offset, dims):
    """Custom access pattern on a DRAM tensor: dims = [[step, count], ...]."""
    return bass.AP(tensor=handle_ap.tensor, offset=offset, ap=list(dims))


def build_nc(n_cores=N_CORES, use_coll=True, stage=9):
    """Build the per-core Bass program (SPMD: same program, sliced inputs)."""
    from contextlib import ExitStack

    nc = bacc.Bacc(num_devices=n_cores)

    x_in = nc.declare_dram_parameter("x", [N_CH, 128, 4096], F16, isOutput=False)
    sel_in = nc.declare_dram_parameter("sel", [128 * COLS], F32, isOutput=False)
    w1t_in = nc.declare_dram_parameter("w1t", [IN_DIM, HID], F16, isOutput=False)
    w2s_in = nc.declare_dram_parameter("w2s", [HID, 32 * 32], F16, isOutput=False)
    b1_in = nc.declare_dram_parameter("b1", [HID], F32, isOutput=False)
    b2_in = nc.declare_dram_parameter("b2", [1], F32, isOutput=False)
    padc_in = nc.declare_dram_parameter("padc", [2], F32, isOutput=False)
    out_t = nc.declare_dram_parameter("out", [128 * COLS], F32, isOutput=True)
    gs_t = nc.declare_dram_parameter("gsums", [2], F32, isOutput=True)

    cc_in = nc.dram_tensor("cc_in", [2], F32)
    cc_out = nc.dram_tensor("cc_out", [2], F32, addr_space="Shared")

    with ExitStack() as ctx:
        tc = ctx.enter_context(tile.TileContext(nc))
        singles = ctx.enter_context(tc.tile_pool(name="singles", bufs=1))
        xx_pool = ctx.enter_context(tc.tile_pool(name="xx", bufs=3))
        ht_pool = ctx.enter_context(tc.tile_pool(name="ht", bufs=MM2_DEFER + 4))
        dv_pool = ctx.enter_context(tc.tile_pool(name="dv", bufs=1))
        ph_pool = ctx.enter_context(tc.tile_pool(name="ph", bufs=2, space="PSUM"))
        ps_pool = ctx.enter_context(tc.tile_pool(name="ps", bufs=1, space="PSUM"))

        # ---- static setup ----------------------------------------------
        # First x chunk before anything else on the HWDGE queue; a small
        # leading slice so the first matmul can start ~5us earlier.
        xx_tiles = {}
        xx_tiles[0] = xx_pool.tile([128, 4096], F16, tag="xx", name="xx")
        nc.sync.dma_start(
            out=xx_tiles[0][:, 0:512],
            in_=_ap(x_in[:], 0, [[4096, 128], [1, 512]]),
        )
        nc.sync.dma_start(
            out=xx_tiles[0][:, 512:4096],
            in_=_ap(x_in[:], 512, [[4096, 128], [1, 3584]]),
        )
        xx_tiles[1] = xx_pool.tile([128, 4096], F16, tag="xx", name="xx")
        nc.sync.dma_start(
            out=xx_tiles[1][:],
            in_=_ap(x_in[:], 128 * 4096, [[4096, 128], [1, 4096]]),
        )

        w1t_sb = singles.tile([128, HID], F16)     # both halves hold W1T
        nc.sync.dma_start(
            out=w1t_sb[:],
            in_=_ap(w1t_in[:], 0, [[0, 2], [HID, IN_DIM], [1, HID]]),
        )
        b1_sb = singles.tile([128, 1], F32)
        nc.sync.dma_start(out=b1_sb[:], in_=_ap(b1_in[:], 0, [[1, HID], [1, 1]]))
        b2_sb = singles.tile([128, 1], F32)
        nc.sync.dma_start(out=b2_sb[:], in_=_ap(b2_in[:], 0, [[0, 128], [1, 1]]))

        # 32 strip matrices [128, 32] fp16, strip c has W2 in column c.
        strips = singles.tile([128, 32, 32], F16)
        nc.sync.dma_start(
            out=strips[:], in_=_ap(w2s_in[:], 0, [[32 * 32, HID], [1, 32 * 32]])
        )
        pc_sb = singles.tile([1, 2], F32)
        nc.sync.dma_start(out=pc_sb[:], in_=_ap(padc_in[:], 0, [[2, 1], [1, 2]]))

        sel_sb = singles.tile([128, COLS], F32)
        nc.sync.dma_start(
            out=sel_sb[:], in_=_ap(sel_in[:], 0, [[COLS, 128], [1, COLS]])
        )

        e_sb = singles.tile([128, COLS], F32)
        out_sb = singles.tile([128, COLS], F32)
        scr = singles.tile([128, Q], F32)
        rr_sel = singles.tile([128, N_ST], F32)
        rr_tot = singles.tile([128, N_ST], F32)
        rr2 = singles.tile([128, 2], F32)
        rr_red = singles.tile([128, 2], F32)
        ones_sb = singles.tile([128, 1], F32)
        tiny = singles.tile([128, 1], F32)
        g2 = singles.tile([1, 2], F32)
        g_sb = singles.tile([128, 2], F32)
        inv = singles.tile([128, 2], F32)
        dinv = singles.tile([128, 1], F32)

        nc.vector.memset(ones_sb[:], 1.0)
        # Zero the unused corner of E (last super-tile has 96 blocks).
        nc.vector.memset(e_sb[96:128, (N_ST - 1) * Q : N_ST * Q], 0.0)
        # Dummy activation: pulls ACT_TABLE_LOAD off the critical path.
        nc.scalar.activation(
            out=tiny[:], in_=ones_sb[:], func=ACTF.Tanh, bias=0.0, scale=1.0
        )
        if use_coll:
            # Warmup AllReduce: pulls the CC-path cold start off the tail
            # and aligns the cores early.
            warm = singles.tile([1, 2], F32)
            cc_win = nc.dram_tensor("cc_win", [2], F32)
            cc_wout = nc.dram_tensor("cc_wout", [2], F32, addr_space="Shared")
            nc.vector.memset(warm[:], 0.0)
            nc.gpsimd.dma_start(out=cc_win[:], in_=warm[0:1, :])
            nc.gpsimd.collective_compute(
                "AllReduce",
                ALU.add,
                replica_groups=[list(range(n_cores))],
                ins=[cc_win[:]],
                outs=[cc_wout[:]],
            )

        # ---- phase 1: mm1 / tanh / mm2 / exp ---------------------------
        # Global mm1 index m: chunk ch=m//16, j=m%16 -> half=j%2, t=j//2.
        # R-order rows [512m, 512(m+1)) = chunk rows half*4096 + 512t ...
        ph = None
        ht = None
        s_ps = None
        next_blk = 0
        cur_tile = 0
        pending_sums = []  # (st, tile_at_exp_emission)

        def emit_st_sums(st):
            # incremental segment sums for super-tile st (DVE)
            nc.vector.tensor_mul(
                scr[:], e_sb[:, st * Q : (st + 1) * Q],
                sel_sb[:, st * Q : (st + 1) * Q],
            )
            nc.vector.reduce_sum(
                rr_sel[:, st : st + 1], scr[:], axis=mybir.AxisListType.X
            )
            nc.vector.reduce_sum(
                rr_tot[:, st : st + 1],
                e_sb[:, st * Q : (st + 1) * Q],
                axis=mybir.AxisListType.X,
            )

        def flush_sums(margin_tile):
            # Only emit DVE sum ops whose exp is comfortably past: keeps
            # the DVE FIFO from blocking tanh chains behind a sum that
            # still waits on ACT/PE progress.
            while pending_sums and pending_sums[0][1] <= margin_tile:
                st, _ = pending_sums.pop(0)
                emit_st_sums(st)

        def emit_mm2_upto(bmax):
            nonlocal s_ps, next_blk
            while next_blk < bmax:
                b = next_blk
                c = b % 32
                g = (b % 128) // 32
                if b % 128 == 0:
                    s_ps = ps_pool.tile([128, Q], F32, tag="score")
                k = b // BLK_PER_HT
                hoff = Q * (b % BLK_PER_HT)
                nc.tensor.matmul(
                    s_ps[32 * g : 32 * g + 32, :],
                    strips[:, c, :],
                    ht_tiles[k][:, hoff : hoff + Q],
                    start=(c == 0),
                    stop=(c == 31),
                    skip_group_check=True,
                    tile_position=(0, 32 * g),
                )
                next_blk += 1
                if next_blk % 128 == 0 or next_blk == N_BLK:
                    st = (next_blk - 1) // 128
                    npart = 128 if next_blk % 128 == 0 else (next_blk - st * 128)
                    nc.scalar.activation(
                        out=e_sb[0:npart, st * Q : (st + 1) * Q],
                        in_=s_ps[0:npart, :],
                        func=ACTF.Exp,
                        bias=b2_sb[0:npart, :],
                        scale=1.0,
                    )
                    pending_sums.append((st, cur_tile))

        ht_tiles = {}
        for m in range(N_MM):
            ch, j = divmod(m, MM_PER_CH)
            if j == 4 and ch + 2 < N_CH and (ch + 2) not in xx_tiles:
                ch2 = ch + 2
                xx_tiles[ch2] = xx_pool.tile([128, 4096], F16, tag="xx", name="xx")
                nc.sync.dma_start(
                    out=xx_tiles[ch2][:],
                    in_=_ap(
                        x_in[:], ch2 * 128 * 4096, [[4096, 128], [1, 4096]]
                    ),
                )
            if j == 0 and ch not in xx_tiles:
                xx_tiles[ch] = xx_pool.tile([128, 4096], F16, tag="xx", name="xx")
                nc.sync.dma_start(
                    out=xx_tiles[ch][:],
                    in_=_ap(
                        x_in[:], ch * 128 * 4096, [[4096, 128], [1, 4096]]
                    ),
                )
            half, t = j % 2, j // 2
            slot = m % PH_MMS
            if slot == 0:
                ph = ph_pool.tile([128, PH_COLS], F32, tag="ph")
            nc.tensor.matmul(
                ph[:, 512 * slot : 512 * (slot + 1)],
                w1t_sb[64 * half : 64 * half + 64, :],
                xx_tiles[ch][64 * half : 64 * half + 64, 512 * t : 512 * (t + 1)],
                start=True,
                stop=True,
            )
            if slot == PH_MMS - 1 or m == N_MM - 1:
                k = m // PH_MMS
                cur_tile = k
                flush_sums(k - 3)
                ncols = 512 * (slot + 1)
                ht_tiles[k] = ht_pool.tile([128, PH_COLS], F16, tag="ht", name="ht")
                hk = ht_tiles[k]
                if k % ASSIST_EVERY == ASSIST_PHASE and ncols == PH_COLS:
                    # DVE rational tanh: z=ph+b1; t=z^2; t4=t^2;
                    # ht = z*(t4 + A2 t + A0) / (t4 + B2 t + B0)
                    zs = dv_pool.tile([128, PH_COLS], F16, tag="dv_zs", name="zs")
                    tq = dv_pool.tile([128, PH_COLS], F16, tag="dv_t", name="tq")
                    t4 = dv_pool.tile([128, PH_COLS], F16, tag="dv_t4", name="t4")
                    nm = dv_pool.tile([128, PH_COLS], F16, tag="dv_nm", name="nm")
                    d32 = dv_pool.tile([128, PH_COLS], F32, tag="dv_d", name="d32")
                    r32 = dv_pool.tile([128, PH_COLS], F32, tag="dv_r", name="r32")
                    with nc.allow_low_precision(reason="f16 rational tanh"):
                        nc.vector.tensor_scalar(
                            out=zs[:], in0=ph[:], scalar1=b1_sb[:],
                            scalar2=None, op0=ALU.add,
                        )
                        nc.vector.tensor_mul(tq[:], zs[:], zs[:])
                        nc.vector.tensor_mul(t4[:], tq[:], tq[:])
                        nc.vector.tensor_scalar(
                            out=nm[:], in0=tq[:], scalar1=float(A2),
                            scalar2=float(A0), op0=ALU.mult, op1=ALU.add,
                        )
                        nc.vector.tensor_add(nm[:], nm[:], t4[:])
                        nc.vector.tensor_mul(nm[:], nm[:], zs[:])
                        nc.vector.tensor_scalar(
                            out=tq[:], in0=tq[:], scalar1=float(B2),
                            scalar2=float(B0), op0=ALU.mult, op1=ALU.add,
                        )
                        nc.vector.tensor_add(d32[:], tq[:], t4[:])
                        nc.vector.reciprocal_approx_fast(out=r32[:], in_=d32[:])
                        nc.vector.tensor_copy(t4[:], r32[:])
                        nc.vector.tensor_mul(hk[:], nm[:], t4[:])
                else:
                    nc.scalar.activation(
                        out=hk[:, 0:ncols],
                        in_=ph[:, 0:ncols],
                        func=ACTF.Tanh,
                        bias=b1_sb[:],
                        scale=1.0,
                    )
                if k >= MM2_DEFER:
                    emit_mm2_upto(((k - MM2_DEFER + 1) * PH_COLS) // Q)
        emit_mm2_upto(N_BLK)
        flush_sums(10**9)

        # ---- segment sums + allreduce ----------------------------------
        if stage <= 1:
            nc.sync.dma_start(
                out=_ap(out_t[:], 0, [[COLS, 128], [1, COLS]]), in_=e_sb[:]
            )
            nc.sync.dma_start(out=gs_t[:], in_=e_sb[0:1, 0:2])
            nc.compile()
            return nc

        # rr_sel/rr_tot [128, 8] -> rr2 [128, 2] -> partition reduce -> [1,2]
        nc.vector.reduce_sum(rr2[:, 0:1], rr_tot[:], axis=mybir.AxisListType.X)
        nc.vector.reduce_sum(rr2[:, 1:2], rr_sel[:], axis=mybir.AxisListType.X)
        ps_rr = ps_pool.tile([128, 2], F32, tag="score")
        nc.tensor.matmul(ps_rr[0:1, :], ones_sb[:], rr2[:], start=True, stop=True)
        nc.scalar.activation(
            out=rr_red[0:1, :], in_=ps_rr[0:1, :], func=ACTF.Copy, bias=0.0,
            scale=1.0,
        )
        # g2 = [sum0, sum1]: sum1 = sel_sum - padc[1];
        #                    sum0 = (tot - sel_sum) - padc[0]
        nc.vector.tensor_sub(g2[0:1, 1:2], rr_red[0:1, 1:2], pc_sb[0:1, 1:2])
        nc.vector.tensor_sub(g2[0:1, 0:1], rr_red[0:1, 0:1], rr_red[0:1, 1:2])
        nc.vector.tensor_sub(g2[0:1, 0:1], g2[0:1, 0:1], pc_sb[0:1, 0:1])

        if use_coll:
            nc.gpsimd.dma_start(out=cc_in[:], in_=g2[0:1, :])
            nc.gpsimd.collective_compute(
                "AllReduce",
                ALU.add,
                replica_groups=[list(range(n_cores))],
                ins=[cc_in[:]],
                outs=[cc_out[:]],
            )
            nc.sync.dma_start(out=gs_t[:], in_=cc_out[:])
            nc.sync.dma_start(
                out=g_sb[:], in_=_ap(cc_out[:], 0, [[0, 128], [1, 2]])
            )
        else:
            nc.sync.dma_start(out=gs_t[:], in_=g2[0:1, :])
            nc.sync.dma_start(
                out=g_sb[:], in_=_ap(gs_t[:], 0, [[0, 128], [1, 2]])
            )

        # ---- normalize + store -----------------------------------------
        nc.vector.reciprocal(out=inv[:], in_=g_sb[:])
        nc.vector.tensor_sub(dinv[:], inv[:, 1:2], inv[:, 0:1])
        nc.vector.tensor_scalar(
            out=out_sb[:],
            in0=sel_sb[:],
            scalar1=dinv[:],
            scalar2=inv[:, 0:1],
            op0=ALU.mult,
            op1=ALU.add,
        )
        nc.vector.tensor_mul(out_sb[:], out_sb[:], e_sb[:])
        nc.sync.dma_start(
            out=_ap(out_t[:], 0, [[COLS, 128], [1, COLS]]), in_=out_sb[:]
        )

    nc.compile()
    return nc


_NC_CACHE = {}


def _get_nc(key=0):
    if key not in _NC_CACHE:
        _NC_CACHE[key] = build_nc()
    return _NC_CACHE[key]


# Host-side index map (identical for every core): for actual row a in
# [0, R_CORE): position in the device (p, col) layout.
_IDX_CACHE = {}


def _layout_index():
    """Return (p, col) arrays mapping core-row a -> device layout slot."""
    if "idx" not in _IDX_CACHE:
        a = np.arange(R_CORE, dtype=np.int64)
        ch, o = a // CH_ROWS, a % CH_ROWS
        half, w = o // 4096, o % 4096
        t, jr = w // 512, w % 512
        m = MM_PER_CH * ch + 2 * t + half
        q = 512 * m + jr
        b, r = q // Q, q % Q
        p = b % 128
        col = (b // 128) * Q + r
        _IDX_CACHE["idx"] = (p, col)
    return _IDX_CACHE["idx"]


def prep_inputs(x, T, W1, b1, W2, b2, n_cores=N_CORES):
    """Host-side shard/layout prep -> per-core input maps."""
    n_rows = x.shape[0]
    assert n_rows == N_ROWS

    x = np.asarray(x, dtype=np.float32)
    xh = np.zeros((N_PAD, IN_DIM), dtype=np.float16)
    xh[:n_rows] = x
    # [n_chunk_tot, 2, 4096, 64] -> [n_chunk_tot, 2, 64, 4096] -> [., 128, 4096]
    n_ch_tot = N_PAD // CH_ROWS
    xd = np.ascontiguousarray(
        xh.reshape(n_ch_tot, 2, 4096, IN_DIM)
        .transpose(0, 1, 3, 2)
        .reshape(n_ch_tot, 128, 4096)
    )

    T = np.asarray(T)
    p_idx, c_idx = _layout_index()

    w1t = np.ascontiguousarray(np.asarray(W1, np.float32).T).astype(np.float16)
    w2s = np.zeros((HID, 32, 32), dtype=np.float16)
    w2v = np.asarray(W2, np.float32).reshape(HID).astype(np.float16)
    for c in range(32):
        w2s[:, c, c] = w2v
    w2s = w2s.reshape(HID, 32 * 32)
    b1h = np.asarray(b1, np.float32).reshape(HID).copy()
    b2h = np.asarray(b2, np.float32).reshape(1).copy()

    # Host estimate of the device's E value on padded rows (x = 0).
    h_pad = np.tanh(b1h)
    e_pad = float(np.exp(b2h[0] + np.asarray(W2, np.float32).reshape(HID) @ h_pad))

    in_maps = []
    for cid in range(n_cores):
        base = cid * R_CORE
        n_real = min(max(n_rows - base, 0), R_CORE)
        selv = np.zeros(R_CORE, dtype=np.float32)
        selv[:n_real] = T[base : base + n_real] == 1
        sel_layout = np.zeros((128, COLS), dtype=np.float32)
        sel_layout[p_idx, c_idx] = selv
        n_pad_c = R_CORE - n_real
        padc = np.array([n_pad_c * e_pad, 0.0], dtype=np.float32)
        in_maps.append(
            {
                "x": xd[cid * N_CH : (cid + 1) * N_CH],
                "sel": sel_layout.reshape(-1),
                "w1t": w1t,
                "w2s": w2s,
                "b1": b1h,
                "b2": b2h,
                "padc": padc,
            }
        )
    return in_maps


def run(x, T, W1, b1, W2, b2, trace=False):
    in_maps = prep_inputs(x, T, W1, b1, W2, b2)
    nc = _get_nc()
    res = run_bass_kernel_spmd(nc, in_maps, list(range(N_CORES)), trace=trace)
    p_idx, c_idx = _layout_index()
    n_rows = x.shape[0]
    out = np.empty(n_rows, dtype=np.float32)
    for cid in range(N_CORES):
        base = cid * R_CORE
        n_real = min(max(n_rows - base, 0), R_CORE)
        if n_real <= 0:
            break
        lay = res.results[cid]["out"].reshape(128, COLS)
        out[base : base + n_real] = lay[p_idx[:n_real], c_idx[:n_real]]
    return out, res


def kernel(x, T, W1, b1, W2, b2):
    out, _ = run(x, T, W1, b1, W2, b2)
    return out
